# revision 10
# baseline (speedup 1.0000x reference)
"""Trainium2 Bass kernel: batched halo gather-rescale (GNN message passing).

Computes, for 11 derived boundary quantities q and 1M halo cells e:
    halos[q, e] = sum_j weights[e, j] * derived[q, src_idx[e, j]]
where derived is built from 12 raw fields:
    derived = [u, v, b_u, b_v, h, hh, dif_h, h+Hb, eta1, min(k_u,k3), min(k_v,k3)]

Distribution strategy: the halo/edge axis E is sharded across the 8
NeuronCores (131072 edges each); the field table is replicated per core in
cell-major [N, 12] layout so every gather is a single contiguous 48-byte row
read. Each (e, j) pair becomes one SWDGE indirect-DMA descriptor; the derived
quantities and the weighted sum are computed on the vector engine per edge.
No cross-core communication is required.

Performance note: the kernel is bound by GpSimd/Q7 descriptor generation.
TRN2 indirect DMA supports exactly one dynamic index per partition per
instruction (128 descriptors), at ~1.4us per instruction sustained — i.e.
~11ns per random row fetched, independent of payload size. 2M (e,j) pairs /
8 cores = 2048 gather instructions/core ~= 2.9ms. The same Q7 floor applies
to the ANT dma_gather/scatter ucode (~9-13ns/idx measured), so sorting/
windowing schemes cannot beat this bound; bulk-streaming the whole table
(~1.1ms/core) lacks any non-Q7 on-chip selection mechanism. The vector-offset
(multi-index) indirect DMA form that would amortize the fixed cost is broken
in the walrus lowering (verified on HW).
"""

import os
import sys

import numpy as np

for _p in ("/opt/trn_rl_repo",):
    if os.path.isdir(_p) and _p not in sys.path:
        sys.path.insert(0, _p)

N_CELLS = 8_388_608
N_Q = 12          # raw field quantities per cell
N_OUT = 11        # derived halo quantities
E_TOTAL = 1_048_576
N_CORES = 8
P = 128           # SBUF partitions
M = 256           # edge pairs per partition per chunk

# q -> column r of the weighted sum S (S_r = w0*A_r + w1*B_r, after the
# min() rows have been folded into columns 9/10 pre-weighting).
_QMAP = ((0, 0), (1, 1), (2, 2), (3, 3), (4, 4), (5, 6), (6, 7),
         (8, 8), (9, 9), (10, 10))  # q=7 is S4 + S5, handled separately


def build_graph(n_cells, e_s, m, d=N_Q):
    """Build the per-core Bass graph (SPMD: identical on all cores)."""
    import concourse.bass as bass
    import concourse.bacc as bacc
    import concourse.mybir as mybir
    from concourse.tile import TileContext

    chunk = P * m
    n_chunks = e_s // chunk
    assert n_chunks * chunk == e_s
    f32 = mybir.dt.float32
    i32 = mybir.dt.int32

    nc = bacc.Bacc("TRN2", target_bir_lowering=False)
    ft = nc.declare_dram_parameter("ft", [n_cells, d], f32, isOutput=False)
    idx = nc.declare_dram_parameter("idx", [2, n_chunks, P, m], i32, isOutput=False)
    w = nc.declare_dram_parameter("w", [2, n_chunks, P, m], f32, isOutput=False)
    out = nc.declare_dram_parameter("out", [N_OUT, e_s], f32, isOutput=True)

    mn = mybir.AluOpType.min

    # HWDGE (sync-engine) DMAs can only encode a small number of semaphore
    # waits; fully buffer the small tiles so their loads never carry WAR waits.
    io_bufs = min(n_chunks, 8)
    with TileContext(nc) as tc:
        with tc.tile_pool(name="io", bufs=io_bufs) as iop, \
             tc.tile_pool(name="gat", bufs=3) as gp, \
             tc.tile_pool(name="ot", bufs=io_bufs) as otp:
            for c in range(n_chunks):
                ia = iop.tile([P, m], i32, tag="ia")
                ib = iop.tile([P, m], i32, tag="ib")
                wa = iop.tile([P, m], f32, tag="wa")
                wb = iop.tile([P, m], f32, tag="wb")
                nc.sync.dma_start(out=ia[:], in_=idx[0, c])
                nc.sync.dma_start(out=ib[:], in_=idx[1, c])
                nc.sync.dma_start(out=wa[:], in_=w[0, c])
                nc.sync.dma_start(out=wb[:], in_=w[1, c])

                A = gp.tile([P, m * d], f32, tag="A")
                B = gp.tile([P, m * d], f32, tag="B")
                # HW indirect DMA supports exactly one dynamic index per
                # partition per instruction: gather 128 rows (one per
                # partition) of d floats at a time.
                for t in range(m):
                    nc.gpsimd.indirect_dma_start(
                        out=A[:, t * d:(t + 1) * d], out_offset=None, in_=ft[:],
                        in_offset=bass.IndirectOffsetOnAxis(ap=ia[:, t:t + 1], axis=0))
                    nc.gpsimd.indirect_dma_start(
                        out=B[:, t * d:(t + 1) * d], out_offset=None, in_=ft[:],
                        in_offset=bass.IndirectOffsetOnAxis(ap=ib[:, t:t + 1], axis=0))

                A3 = A[:].rearrange("p (m d) -> p m d", d=d)
                B3 = B[:].rearrange("p (m d) -> p m d", d=d)
                # min(k_u, k3), min(k_v, k3) must be taken before weighting
                nc.vector.tensor_tensor(out=A3[:, :, 9], in0=A3[:, :, 9], in1=A3[:, :, 11], op=mn)
                nc.vector.tensor_tensor(out=A3[:, :, 10], in0=A3[:, :, 10], in1=A3[:, :, 11], op=mn)
                nc.vector.tensor_tensor(out=B3[:, :, 9], in0=B3[:, :, 9], in1=B3[:, :, 11], op=mn)
                nc.vector.tensor_tensor(out=B3[:, :, 10], in0=B3[:, :, 10], in1=B3[:, :, 11], op=mn)
                # S = w0 * A + w1 * B (weights broadcast along the field axis)
                wab = wa[:].unsqueeze(2).to_broadcast([P, m, d])
                wbb = wb[:].unsqueeze(2).to_broadcast([P, m, d])
                nc.vector.tensor_mul(out=A3, in0=A3, in1=wab)
                nc.vector.tensor_mul(out=B3, in0=B3, in1=wbb)
                nc.vector.tensor_add(out=A3, in0=A3, in1=B3)

                O = otp.tile([P, N_OUT * m], f32, tag="O")
                O3 = O[:].rearrange("p (q m) -> p q m", m=m)
                for q, r in _QMAP:
                    nc.vector.tensor_copy(out=O3[:, q], in_=A3[:, :, r])
                nc.vector.tensor_add(out=O3[:, 7], in0=A3[:, :, 4], in1=A3[:, :, 5])

                dst = out[:].rearrange("q (c p m) -> c p q m", p=P, m=m)[c]
                nc.sync.dma_start(out=dst, in_=O3)
    nc.finalize()
    return nc


_GRAPH_CACHE = {}


def _get_graph():
    if "full" not in _GRAPH_CACHE:
        _GRAPH_CACHE["full"] = build_graph(N_CELLS, E_TOTAL // N_CORES, M)
    return _GRAPH_CACHE["full"]


def kernel(fields, src_idx, weights):
    from concourse.bass_utils import run_bass_kernel_spmd

    e_s = E_TOTAL // N_CORES
    n_chunks = e_s // (P * M)

    ft = np.ascontiguousarray(np.asarray(fields, dtype=np.float32).T)
    si = np.asarray(src_idx, dtype=np.int32)
    wt = np.asarray(weights, dtype=np.float32)

    in_maps = []
    for i in range(N_CORES):
        sl = slice(i * e_s, (i + 1) * e_s)
        idx_i = np.ascontiguousarray(si[sl].T).reshape(2, n_chunks, P, M)
        w_i = np.ascontiguousarray(wt[sl].T).reshape(2, n_chunks, P, M)
        in_maps.append({"ft": ft, "idx": idx_i, "w": w_i})

    nc = _get_graph()
    trace = bool(int(os.environ.get("KERNEL_TRACE", "0")))
    if trace:
        try:
            import profhook
            profhook.install()
        except Exception as e:
            print(f"profile hook unavailable ({e}); running untraced")
            trace = False
    res = run_bass_kernel_spmd(nc, in_maps, core_ids=list(range(N_CORES)),
                               trace=trace)
    if trace and res.exec_time_ns is not None:
        print(f"HW exec time: {res.exec_time_ns} ns")
    return np.concatenate([res.results[i]["out"] for i in range(N_CORES)],
                          axis=1)
